# revision 11
# baseline (speedup 1.0000x reference)
"""Trainium2 Bass kernel for the GRU greedy decoder (nn_Decoder).

Strategy (8 NeuronCores):
  - W_out is vocab-sharded: each core keeps a [1024, 4000] slice of W_out.T
    resident in SBUF and computes logits for its 4000-vocab shard with the
    batch as the PE stationary operand (col-tiled: partitions 0:64 compute
    vocab [0,2000), partitions 64:128 compute [2000,4000)).
  - The GRU is hidden-sharded: core c computes gates for hidden units
    [128c, 128c+128) and the per-step hidden state is AllGathered
    (pre-transposed so it lands ready as the matmul stationary operand).
  - Greedy argmax + log_softmax stats (per-half max / sumexp / argmax index)
    are exchanged with a tiny AllGather each step and combined identically on
    every core; sigmoid is computed as tanh so every activation lives in one
    ACT function table.
  - The embedding lookup is an indirect DMA row gather using the token ids.
All matmul math is fp32 (the greedy token chain requires it).
"""

import sys

for _p in ("/opt/trn_rl_repo", "/root/.axon_site/_ro/trn_rl_repo"):
    if _p not in sys.path:
        sys.path.insert(0, _p)

import os
import numpy as np

import concourse.bass as bass
import concourse.bacc as bacc
import concourse.tile as tile
import concourse.mybir as mybir
from concourse import bass_utils
from concourse.bass import IndirectOffsetOnAxis

F32 = mybir.dt.float32
U32 = mybir.dt.uint32
U8 = mybir.dt.uint8
AF = mybir.ActivationFunctionType
ALU = mybir.AluOpType
AX = mybir.AxisListType

V, H, B = 32000, 1024, 64
T = int(os.environ.get("DEC_T", "64"))      # decode steps (64 real)
REPS = int(os.environ.get("DEC_REPS", "1")) # repeat whole decode (timing only)
NOCC = os.environ.get("DEC_NOCC", "0") == "1"  # replace collectives w/ local DMA
NCORE = 8
VS = V // NCORE          # 4000 vocab per core
HS = H // NCORE          # 128 hidden units per core
GS = 3 * HS              # 384 gate rows per core
KC = H // 128            # 8 contraction chunks
VCH = 500                # vocab chunk per PSUM bank
HALF = 2000              # vocab per partition-half
SOS = 1

_CACHE = {}


def _build():
    nc = bacc.Bacc("TRN2", target_bir_lowering=False, debug=False, num_devices=NCORE)

    def inp(name, shape, dtype=F32):
        return nc.dram_tensor(name, list(shape), dtype, kind="ExternalInput")

    emb_t = inp("emb", [V, H])
    wot_t = inp("wot", [128, KC, VS])
    wih_t = inp("wih", [128, KC, GS])
    whh_t = inp("whh", [128, KC, GS])
    bo_t = inp("bo", [1, VS])
    brz_t = inp("brz", [1, 2 * HS])
    bin_t = inp("bin", [1, HS])
    bhn_t = inp("bhn", [1, HS])
    h0sl_t = inp("h0sl", [B, HS])
    h0T_t = inp("h0T", [128, KC, B])
    x0T_t = inp("x0T", [128, KC, B])
    ident_t = inp("ident", [128, 128])
    voff_t = inp("voff", [128, 1])

    out_logp = nc.dram_tensor("out_logp", [B, T, VS], F32, kind="ExternalOutput")
    out_hsl = nc.dram_tensor("out_hsl", [B, HS], F32, kind="ExternalOutput")

    RG = [list(range(NCORE))]

    with tile.TileContext(nc) as tc:
        with tc.tile_pool(name="wp", bufs=1) as wp, \
             tc.tile_pool(name="sp", bufs=2) as sp, \
             tc.tile_pool(name="pp", bufs=1, space="PSUM") as pp, \
             tc.tile_pool(name="dp", bufs=2, space="DRAM") as dp:

            # ---- resident weights / constants ------------------------------
            wot = wp.tile([128, KC, VS], F32, name="wot_sb")
            nc.sync.dma_start(wot[:], wot_t[:, :, :])
            wih = wp.tile([128, KC, GS], F32, name="wih_sb")
            nc.sync.dma_start(wih[:], wih_t[:, :, :])
            whh = wp.tile([128, KC, GS], F32, name="whh_sb")
            nc.sync.dma_start(whh[:], whh_t[:, :, :])
            bo = wp.tile([1, VS], F32, name="bo_sb")
            nc.sync.dma_start(bo[:], bo_t[:, :])
            brz = wp.tile([1, 2 * HS], F32, name="brz_sb")
            nc.sync.dma_start(brz[:], brz_t[:, :])
            bin_ = wp.tile([1, HS], F32, name="bin_sb")
            nc.sync.dma_start(bin_[:], bin_t[:, :])
            bhn = wp.tile([1, HS], F32, name="bhn_sb")
            nc.sync.dma_start(bhn[:], bhn_t[:, :])
            ident = wp.tile([128, 128], F32, name="ident_sb")
            nc.sync.dma_start(ident[:], ident_t[:, :])
            voff = wp.tile([128, 1], F32, name="voff_sb")
            nc.sync.dma_start(voff[:], voff_t[:, :])
            x0T = wp.tile([128, KC, B], F32, name="x0T_sb")
            nc.sync.dma_start(x0T[:], x0T_t[:, :, :])
            ones = wp.tile([1, B], F32, name="ones_sb")
            nc.vector.memset(ones[:], 1.0)
            bigc = wp.tile([128, 2 * NCORE], F32, name="bigc_sb")
            nc.vector.memset(bigc[:], 1.0e9)
            dumm = wp.tile([1, 1], F32, name="dumm_sb")
            nc.vector.memset(dumm[:], 0.0)

            h_sl0 = wp.tile([B, HS], F32, name="h_sl0")
            nc.sync.dma_start(h_sl0[:], h0sl_t[:, :])
            hT0 = wp.tile([128, KC, B], F32, name="hT0")
            nc.sync.dma_start(hT0[:], h0T_t[:, :, :])

            dma_engs = [nc.sync, nc.scalar, nc.gpsimd]

            def all_gather(cin, cout):
                if NOCC:
                    nc.gpsimd.dma_start(cout[0:cin.shape[0], :], cin[:])
                else:
                    nc.gpsimd.collective_compute(
                        "AllGather", ALU.bypass, replica_groups=RG,
                        ins=[cin[:]], outs=[cout[:]])

            def writeback(prev, su):
                p_lg, p_allst, p_Mt, p_t = prev
                NE = 2 * NCORE
                dd16 = sp.tile([128, NE], F32, tag="dd16", name=f"dd16{su}")
                nc.vector.tensor_scalar(
                    dd16[:].rearrange("b (c f) -> b c f", f=1),
                    p_allst[:, :, 0:1], p_Mt[:], None, op0=ALU.subtract)
                ee16 = sp.tile([128, NE], F32, tag="ee16", name=f"ee16{su}")
                nc.scalar.activation(ee16[:], dd16[:], AF.Exp)
                tt16 = sp.tile([128, NE], F32, tag="tt16", name=f"tt16{su}")
                nc.vector.tensor_tensor(
                    tt16[:].rearrange("b (c f) -> b c f", f=1),
                    ee16[:].rearrange("b (c f) -> b c f", f=1),
                    p_allst[:, :, 1:2], ALU.mult)
                St = sp.tile([128, 1], F32, tag="St", name=f"St{su}")
                nc.vector.reduce_sum(St[:], tt16[:], axis=AX.X)
                lnS = sp.tile([128, 1], F32, tag="lnS", name=f"lnS{su}")
                nc.scalar.activation(lnS[:], St[:], AF.Ln)
                # dummy tanh swaps the ACT table back while off-critical
                nc.scalar.activation(dumm[:], dumm[:], AF.Tanh)
                lse = sp.tile([128, 1], F32, tag="lse", name=f"lse{su}")
                nc.vector.tensor_tensor(lse[:], p_Mt[:], lnS[:], ALU.add)
                outb = sp.tile([128, HALF], F32, tag="outb", bufs=1,
                               name=f"outb{su}")
                for q in range(4):
                    cs = slice(VCH * q, VCH * (q + 1))
                    nc.gpsimd.tensor_scalar(outb[:, cs], p_lg[:, cs],
                                            lse[:], None, op0=ALU.subtract)
                    dma_engs[q % 3].dma_start(
                        out_logp[:, p_t:p_t + 1, VCH * q:VCH * (q + 1)],
                        outb[0:B, cs].rearrange("b (x v) -> b x v", x=1))
                    dma_engs[(q + 1) % 3].dma_start(
                        out_logp[:, p_t:p_t + 1,
                                 HALF + VCH * q:HALF + VCH * (q + 1)],
                        outb[B:128, cs].rearrange("b (x v) -> b x v", x=1))

            prev = None
            h_sl, hT = h_sl0, hT0
            x_lo = x_hi = None

            for rep in range(REPS):
                if rep > 0:
                    h_sl, hT = h_sl0, hT0
                for t in range(T):
                    first = t == 0
                    su = f"{rep}_{t}"
                    want_tok = (t < T - 1) or (rep < REPS - 1)

                    # ==== GRU =============================================
                    rz_ps = pp.tile([B, 2 * HS], F32, tag="rz", name=f"rz{su}")
                    gin_ps = pp.tile([B, HS], F32, tag="gin", name=f"gin{su}")
                    ghn_ps = pp.tile([B, HS], F32, tag="ghn", name=f"ghn{su}")
                    nc.tensor.matmul(rz_ps[:], ones[:], brz[:], start=True,
                                     stop=False)
                    nc.tensor.matmul(gin_ps[:], ones[:], bin_[:], start=True,
                                     stop=False)
                    nc.tensor.matmul(ghn_ps[:], ones[:], bhn[:], start=True,
                                     stop=False)
                    for k in range(KC):
                        nc.tensor.matmul(rz_ps[:], hT[:, k, :],
                                         whh[:, k, 0:2 * HS],
                                         start=False, stop=False)
                        nc.tensor.matmul(ghn_ps[:], hT[:, k, :],
                                         whh[:, k, 2 * HS:GS],
                                         start=False, stop=(k == KC - 1))
                    for k in range(KC):
                        last = k == KC - 1
                        if first:
                            src = x0T[:, k, :]
                        else:
                            xsrc = x_lo if k < 4 else x_hi
                            xcol = (k % 4) * 128
                            xp_ps = pp.tile([128, B], F32, tag="xp", bufs=2,
                                            name=f"xp{su}_{k}")
                            nc.tensor.transpose(xp_ps[:],
                                                xsrc[:, xcol:xcol + 128],
                                                ident[0:B, 0:B])
                            xt_sb = sp.tile([128, B], F32, tag="xt", bufs=2,
                                            name=f"xt{su}_{k}")
                            nc.vector.tensor_copy(xt_sb[:], xp_ps[:])
                            src = xt_sb[:]
                        nc.tensor.matmul(rz_ps[:], src, wih[:, k, 0:2 * HS],
                                         start=False, stop=last)
                        nc.tensor.matmul(gin_ps[:], src, wih[:, k, 2 * HS:GS],
                                         start=False, stop=last)

                    # gates: sigmoid(v) = 0.5*(1+tanh(v/2)) -> all-tanh ACT
                    rz_sb = sp.tile([B, 2 * HS], F32, tag="rzsb", name=f"rzsb{su}")
                    nc.scalar.activation(rz_sb[:], rz_ps[:], AF.Tanh, scale=0.5)
                    t1 = sp.tile([B, HS], F32, tag="t1", name=f"t1_{su}")
                    nc.vector.scalar_tensor_tensor(t1[:], rz_sb[:, 0:HS], 1.0,
                                                   ghn_ps[:], op0=ALU.add,
                                                   op1=ALU.mult)
                    t2 = sp.tile([B, HS], F32, tag="t2", name=f"t2_{su}")
                    nc.vector.scalar_tensor_tensor(t2[:], t1[:], 0.5, gin_ps[:],
                                                   op0=ALU.mult, op1=ALU.add)
                    n_sb = sp.tile([B, HS], F32, tag="nsb", name=f"nsb{su}")
                    nc.scalar.activation(n_sb[:], t2[:], AF.Tanh)
                    t3 = sp.tile([B, HS], F32, tag="t3", name=f"t3_{su}")
                    nc.vector.tensor_tensor(t3[:], h_sl[:], n_sb[:], ALU.subtract)
                    t4 = sp.tile([B, HS], F32, tag="t4", name=f"t4_{su}")
                    nc.vector.scalar_tensor_tensor(t4[:], rz_sb[:, HS:2 * HS],
                                                   1.0, t3[:], op0=ALU.add,
                                                   op1=ALU.mult)
                    h_new = sp.tile([B, HS], F32, tag="hsl", name=f"hsl{su}")
                    nc.vector.scalar_tensor_tensor(h_new[:], t4[:], 0.5, n_sb[:],
                                                   op0=ALU.mult, op1=ALU.add)
                    h_sl = h_new

                    # ==== AllGather h (transposed) ========================
                    hxp_ps = pp.tile([128, B], F32, tag="xp", bufs=2,
                                     name=f"hxp{su}")
                    nc.tensor.transpose(hxp_ps[:], h_sl[:], ident[0:B, 0:B])
                    hTo = sp.tile([128, B], F32, tag="hTo", name=f"hTo{su}")
                    nc.vector.tensor_copy(hTo[:], hxp_ps[:])
                    ch_in = dp.tile([128, B], F32, tag="chin", name=f"chin{su}")
                    nc.sync.dma_start(ch_in[:], hTo[:])
                    ch_out = dp.tile([KC * 128, B], F32, tag="chout",
                                     addr_space="Shared", name=f"chout{su}")
                    all_gather(ch_in, ch_out)
                    hT_new = sp.tile([128, KC, B], F32, tag="hT", name=f"hT{su}")
                    for k in range(KC):
                        dma_engs[k % 3].dma_start(
                            hT_new[:, k, :], ch_out[128 * k:128 * (k + 1), :])
                    hT = hT_new

                    # ==== deferred log_softmax + writeback of step t-1 ====
                    if prev is not None:
                        writeback(prev, su)

                    # ==== projection (col-tiled, 2 passes of 2 chunks) ====
                    lg = sp.tile([128, HALF], F32, tag="lg", bufs=1,
                                 name=f"lg{su}")
                    mxs, idxs = [], []
                    for p in range(2):
                        js = (2 * p, 2 * p + 1)
                        tiles = {}
                        for j in js:
                            pj = pp.tile([128, VCH], F32, tag=f"pj{j % 2}",
                                         name=f"pj{su}_{j}")
                            tiles[j] = pj
                            nc.tensor.matmul(pj[0:B, :], ones[:],
                                             bo[:, VCH * j:VCH * (j + 1)],
                                             start=True, stop=False,
                                             tile_position=(0, 0))
                            nc.tensor.matmul(
                                pj[B:128, :], ones[:],
                                bo[:, HALF + VCH * j:HALF + VCH * (j + 1)],
                                start=True, stop=False, tile_position=(0, 64))
                        for k in range(KC):
                            last = k == KC - 1
                            for j in js:
                                pj = tiles[j]
                                nc.tensor.matmul(
                                    pj[0:B, :], hT[:, k, :],
                                    wot[:, k, VCH * j:VCH * (j + 1)],
                                    start=False, stop=last,
                                    tile_position=(0, 0))
                                nc.tensor.matmul(
                                    pj[B:128, :], hT[:, k, :],
                                    wot[:, k, HALF + VCH * j:HALF + VCH * (j + 1)],
                                    start=False, stop=last,
                                    tile_position=(0, 64))
                        for i, j in enumerate(js):
                            if i == 0:
                                nc.scalar.copy(lg[:, VCH * j:VCH * (j + 1)],
                                               tiles[j][:])
                            else:
                                nc.vector.tensor_copy(
                                    lg[:, VCH * j:VCH * (j + 1)], tiles[j][:])
                        # per-pass top8(+index): pass-0 stats hide under
                        # pass-1 matmuls
                        mx = sp.tile([128, 8], F32, tag=f"mx{p}",
                                     name=f"mx{su}_{p}")
                        nc.vector.max(mx[:], lg[:, 1000 * p:1000 * (p + 1)])
                        mxs.append(mx)
                        if want_tok:
                            mi = sp.tile([128, 8], U32, tag=f"mi{p}",
                                         name=f"mi{su}_{p}")
                            nc.vector.max_index(mi[:], mx[:],
                                                lg[:, 1000 * p:1000 * (p + 1)])
                            idxs.append(mi)

                    # ==== local stats =====================================
                    stats = sp.tile([128, 4], F32, tag="stats", name=f"stats{su}")
                    nc.vector.tensor_tensor(stats[:, 0:1], mxs[0][:, 0:1],
                                            mxs[1][:, 0:1], ALU.max)
                    neg_m = sp.tile([128, 1], F32, tag="negm", name=f"negm{su}")
                    nc.vector.tensor_scalar_mul(neg_m[:], stats[:, 0:1], -1.0)
                    s4 = sp.tile([128, 4], F32, tag="s4", name=f"s4_{su}")
                    for j in range(4):
                        sink = pp.tile([128, VCH], F32, tag=f"pj{j % 2}",
                                       name=f"esink{su}_{j}")
                        nc.scalar.activation(sink[:], lg[:, VCH * j:VCH * (j + 1)],
                                             AF.Exp, bias=neg_m[:],
                                             accum_out=s4[:, j:j + 1])
                    nc.vector.reduce_sum(stats[:, 1:2], s4[:], axis=AX.X)
                    if want_tok:
                        i0 = sp.tile([128, 1], F32, tag="i0", name=f"i0{su}")
                        nc.vector.tensor_copy(i0[:], idxs[0][:, 0:1])
                        i1 = sp.tile([128, 1], F32, tag="i1", name=f"i1{su}")
                        nc.vector.tensor_copy(i1[:], idxs[1][:, 0:1])
                        nc.vector.tensor_scalar_add(i1[:], i1[:], 1000.0)
                        gtm = sp.tile([128, 1], U8, tag="gtm", name=f"gtm{su}")
                        nc.vector.tensor_tensor(gtm[:], mxs[1][:, 0:1],
                                                mxs[0][:, 0:1], ALU.is_gt)
                        nc.vector.select(stats[:, 2:3], gtm[:], i1[:], i0[:])
                        nc.vector.tensor_tensor(stats[:, 2:3], stats[:, 2:3],
                                                voff[:], ALU.add)

                    # ==== exchange stats ==================================
                    cs_in = dp.tile([128, 4], F32, tag="csin", name=f"csin{su}")
                    nc.sync.dma_start(cs_in[:], stats[:])
                    cs_out = dp.tile([NCORE * 128, 4], F32, tag="csout",
                                     addr_space="Shared", name=f"csout{su}")
                    all_gather(cs_in, cs_out)
                    allst = sp.tile([128, 2 * NCORE, 4], F32, tag="allst",
                                    name=f"allst{su}")
                    rd = cs_out[:].rearrange("(c h b) f -> b (c h) f",
                                             c=NCORE, h=2)
                    nc.sync.dma_start(allst[0:B, :, :], rd)
                    nc.scalar.dma_start(allst[B:128, :, :], rd)

                    # ==== global max + winner token =======================
                    NE = 2 * NCORE
                    Mt = sp.tile([128, 1], F32, tag="Mt", name=f"Mt{su}")
                    nc.vector.tensor_reduce(Mt[:], allst[:, :, 0:1],
                                            axis=AX.XY, op=ALU.max)
                    if want_tok:
                        mask16 = sp.tile([128, NE], U8, tag="mask16",
                                         name=f"mask16{su}")
                        nc.vector.tensor_scalar(
                            mask16[:].rearrange("b (c f) -> b c f", f=1),
                            allst[:, :, 0:1], Mt[:], None, op0=ALU.is_equal)
                        c16 = sp.tile([128, NE], F32, tag="c16", name=f"c16{su}")
                        nc.vector.select(
                            c16[:].rearrange("b (c f) -> b c f", f=1),
                            mask16[:].rearrange("b (c f) -> b c f", f=1),
                            allst[:, :, 2:3],
                            bigc[:].rearrange("b (c f) -> b c f", f=1))
                        tokf = sp.tile([128, 1], F32, tag="tokf", name=f"tokf{su}")
                        nc.vector.tensor_reduce(tokf[:], c16[:], axis=AX.X,
                                                op=ALU.min)
                        tokf2 = sp.tile([128, 1], F32, tag="tokf2",
                                        name=f"tokf2{su}")
                        nc.vector.tensor_scalar_mul(tokf2[:], tokf[:], 2.0)
                        tok_u = sp.tile([128, 1], U32, tag="toku",
                                        name=f"toku{su}")
                        nc.vector.tensor_copy(tok_u[:], tokf2[:])
                        # emb viewed as [2V, 512]: row 2t = first half of
                        # token t's embedding, row 2t+1 = second half
                        emb2 = emb_t[:, :].rearrange("v (a c) -> (v a) c", a=2)
                        x_lo = sp.tile([B, H // 2], F32, tag="xlo", bufs=1,
                                       name=f"xlo{su}")
                        nc.gpsimd.indirect_dma_start(
                            x_lo[:], None, emb2,
                            IndirectOffsetOnAxis(ap=tok_u[0:B, :], axis=0),
                            bounds_check=2 * V - 1, oob_is_err=False)
                        x_hi = sp.tile([B, H // 2], F32, tag="xhi", bufs=1,
                                       name=f"xhi{su}")
                        nc.gpsimd.indirect_dma_start(
                            x_hi[:], None, emb2,
                            IndirectOffsetOnAxis(ap=tok_u[0:B, :], axis=0),
                            element_offset=H // 2,
                            bounds_check=2 * V - 1, oob_is_err=False)

                    prev = (lg, allst, Mt, t)

            # ---- final writeback + hidden state ---------------------------
            writeback(prev, "F")
            nc.sync.dma_start(out_hsl[:, :], h_sl[:])

    nc.compile()
    return nc


def _prep_core_inputs(c, encoder_hidden, emb, W_ih, W_hh, b_ih, b_hh, W_out,
                      b_out):
    HSl = slice(c * HS, (c + 1) * HS)
    gate_rows = np.r_[c * HS:(c + 1) * HS,
                      H + c * HS:H + (c + 1) * HS,
                      2 * H + c * HS:2 * H + (c + 1) * HS]
    rz_rows = gate_rows[:2 * HS]
    n_rows = gate_rows[2 * HS:]

    def to_kpb(a):  # [rows, H] -> [128, KC, rows] stationary layout
        return np.ascontiguousarray(
            a.T.reshape(KC, 128, a.shape[0]).transpose(1, 0, 2))

    h0 = encoder_hidden[0]
    x0 = np.broadcast_to(emb[SOS], (B, H))
    return {
        "emb": np.ascontiguousarray(emb),
        "wot": to_kpb(W_out[c * VS:(c + 1) * VS]),
        "wih": to_kpb(W_ih[gate_rows]),
        "whh": to_kpb(W_hh[gate_rows]),
        "bo": b_out[c * VS:(c + 1) * VS][None, :],
        "brz": (b_ih[rz_rows] + b_hh[rz_rows])[None, :],
        "bin": b_ih[n_rows][None, :],
        "bhn": b_hh[n_rows][None, :],
        "h0sl": np.ascontiguousarray(h0[:, HSl]),
        "h0T": np.ascontiguousarray(h0.T.reshape(KC, 128, B).transpose(1, 0, 2)),
        "x0T": np.ascontiguousarray(x0.T.reshape(KC, 128, B).transpose(1, 0, 2)),
        "ident": np.eye(128, dtype=np.float32),
        "voff": (c * VS + 2000.0 * (np.arange(128) >= 64)).astype(np.float32)[:, None],
    }


def kernel(encoder_outputs, encoder_hidden, emb, W_ih, W_hh, b_ih, b_hh,
           W_out, b_out, _trace=False):
    del encoder_outputs  # unused by the reference decoder (no attention)
    args = [np.asarray(a, dtype=np.float32) for a in
            (encoder_hidden, emb, W_ih, W_hh, b_ih, b_hh, W_out, b_out)]

    if "nc" not in _CACHE:
        _CACHE["nc"] = _build()
    nc = _CACHE["nc"]

    in_maps = [_prep_core_inputs(c, *args) for c in range(NCORE)]
    res = bass_utils.run_bass_kernel_spmd(
        nc, in_maps, core_ids=list(range(NCORE)), trace=_trace)

    log_probs = np.concatenate(
        [res.results[c]["out_logp"] for c in range(NCORE)], axis=2)
    hidden = np.concatenate(
        [res.results[c]["out_hsl"] for c in range(NCORE)], axis=1)[None]
    if _trace:
        _CACHE["last_result"] = res
    return log_probs, hidden


# revision 12
# speedup vs baseline: 1.0833x; 1.0833x over previous
"""Trainium2 Bass kernel for the GRU greedy decoder (nn_Decoder).

Strategy (8 NeuronCores):
  - W_out is vocab-sharded: each core keeps a [1024, 4000] slice of W_out.T
    resident in SBUF and computes logits for its 4000-vocab shard with the
    batch as the PE stationary operand (col-tiled: partitions 0:64 compute
    vocab [0,2000), partitions 64:128 compute [2000,4000)).
  - The GRU is hidden-sharded: core c computes gates for hidden units
    [128c, 128c+128) and the per-step hidden state is AllGathered
    (pre-transposed so it lands ready as the matmul stationary operand).
  - Greedy argmax + log_softmax stats (per-half max / sumexp / argmax index)
    are exchanged with a tiny AllGather each step and combined identically on
    every core; sigmoid is computed as tanh so every activation lives in one
    ACT function table.
  - The embedding lookup is an indirect DMA row gather using the token ids.
All matmul math is fp32 (the greedy token chain requires it).
"""

import sys

for _p in ("/opt/trn_rl_repo", "/root/.axon_site/_ro/trn_rl_repo"):
    if _p not in sys.path:
        sys.path.insert(0, _p)

import os
import numpy as np

import concourse.bass as bass
import concourse.bacc as bacc
import concourse.tile as tile
import concourse.mybir as mybir
from concourse import bass_utils
from concourse.bass import IndirectOffsetOnAxis

F32 = mybir.dt.float32
U32 = mybir.dt.uint32
U8 = mybir.dt.uint8
AF = mybir.ActivationFunctionType
ALU = mybir.AluOpType
AX = mybir.AxisListType

V, H, B = 32000, 1024, 64
T = int(os.environ.get("DEC_T", "64"))      # decode steps (64 real)
REPS = int(os.environ.get("DEC_REPS", "1")) # repeat whole decode (timing only)
NOCC = os.environ.get("DEC_NOCC", "0") == "1"  # replace collectives w/ local DMA
NOPROJ = os.environ.get("DEC_NOPROJ", "0") == "1"  # skip proj k-loop (timing expt)
SPLITPJ = os.environ.get("DEC_SPLITPJ", "0") == "1"  # separate A/B psum tiles
NCORE = 8
VS = V // NCORE          # 4000 vocab per core
HS = H // NCORE          # 128 hidden units per core
GS = 3 * HS              # 384 gate rows per core
KC = H // 128            # 8 contraction chunks
VCH = 500                # vocab chunk per PSUM bank
HALF = 2000              # vocab per partition-half
SOS = 1

_CACHE = {}


def _build():
    nc = bacc.Bacc("TRN2", target_bir_lowering=False, debug=False, num_devices=NCORE)

    def inp(name, shape, dtype=F32):
        return nc.dram_tensor(name, list(shape), dtype, kind="ExternalInput")

    emb_t = inp("emb", [V, H])
    wot_t = inp("wot", [128, KC, VS])
    wih_t = inp("wih", [128, KC, GS])
    whh_t = inp("whh", [128, KC, GS])
    bo_t = inp("bo", [1, VS])
    brz_t = inp("brz", [1, 2 * HS])
    bin_t = inp("bin", [1, HS])
    bhn_t = inp("bhn", [1, HS])
    h0sl_t = inp("h0sl", [B, HS])
    h0T_t = inp("h0T", [128, KC, B])
    x0T_t = inp("x0T", [128, KC, B])
    ident_t = inp("ident", [128, 128])
    voff_t = inp("voff", [128, 1])

    out_logp = nc.dram_tensor("out_logp", [B, T, VS], F32, kind="ExternalOutput")
    out_hsl = nc.dram_tensor("out_hsl", [B, HS], F32, kind="ExternalOutput")

    RG = [list(range(NCORE))]

    with tile.TileContext(nc) as tc:
        with tc.tile_pool(name="wp", bufs=1) as wp, \
             tc.tile_pool(name="sp", bufs=2) as sp, \
             tc.tile_pool(name="pp", bufs=1, space="PSUM") as pp, \
             tc.tile_pool(name="dp", bufs=2, space="DRAM") as dp:

            # ---- resident weights / constants ------------------------------
            wot = wp.tile([128, KC, VS], F32, name="wot_sb")
            nc.sync.dma_start(wot[:], wot_t[:, :, :])
            wih = wp.tile([128, KC, GS], F32, name="wih_sb")
            nc.sync.dma_start(wih[:], wih_t[:, :, :])
            whh = wp.tile([128, KC, GS], F32, name="whh_sb")
            nc.sync.dma_start(whh[:], whh_t[:, :, :])
            bo = wp.tile([1, VS], F32, name="bo_sb")
            nc.sync.dma_start(bo[:], bo_t[:, :])
            brz = wp.tile([1, 2 * HS], F32, name="brz_sb")
            nc.sync.dma_start(brz[:], brz_t[:, :])
            bin_ = wp.tile([1, HS], F32, name="bin_sb")
            nc.sync.dma_start(bin_[:], bin_t[:, :])
            bhn = wp.tile([1, HS], F32, name="bhn_sb")
            nc.sync.dma_start(bhn[:], bhn_t[:, :])
            ident = wp.tile([128, 128], F32, name="ident_sb")
            nc.sync.dma_start(ident[:], ident_t[:, :])
            voff = wp.tile([128, 1], F32, name="voff_sb")
            nc.sync.dma_start(voff[:], voff_t[:, :])
            x0T = wp.tile([128, KC, B], F32, name="x0T_sb")
            nc.sync.dma_start(x0T[:], x0T_t[:, :, :])
            ones = wp.tile([1, B], F32, name="ones_sb")
            nc.vector.memset(ones[:], 1.0)
            bigc = wp.tile([128, 2 * NCORE], F32, name="bigc_sb")
            nc.vector.memset(bigc[:], 1.0e9)
            dumm = wp.tile([1, 1], F32, name="dumm_sb")
            nc.vector.memset(dumm[:], 0.0)

            h_sl0 = wp.tile([B, HS], F32, name="h_sl0")
            nc.sync.dma_start(h_sl0[:], h0sl_t[:, :])
            hT0 = wp.tile([128, KC, B], F32, name="hT0")
            nc.sync.dma_start(hT0[:], h0T_t[:, :, :])

            dma_engs = [nc.sync, nc.scalar, nc.gpsimd]

            def all_gather(cin, cout):
                if NOCC:
                    nc.gpsimd.dma_start(cout[0:cin.shape[0], :], cin[:])
                else:
                    nc.gpsimd.collective_compute(
                        "AllGather", ALU.bypass, replica_groups=RG,
                        ins=[cin[:]], outs=[cout[:]])

            def writeback(prev, su):
                p_lg, p_allst, p_Mt, p_t = prev
                NE = 2 * NCORE
                dd16 = sp.tile([128, NE], F32, tag="dd16", name=f"dd16{su}")
                nc.vector.tensor_scalar(
                    dd16[:].rearrange("b (c f) -> b c f", f=1),
                    p_allst[:, :, 0:1], p_Mt[:], None, op0=ALU.subtract)
                ee16 = sp.tile([128, NE], F32, tag="ee16", name=f"ee16{su}")
                nc.scalar.activation(ee16[:], dd16[:], AF.Exp)
                tt16 = sp.tile([128, NE], F32, tag="tt16", name=f"tt16{su}")
                nc.vector.tensor_tensor(
                    tt16[:].rearrange("b (c f) -> b c f", f=1),
                    ee16[:].rearrange("b (c f) -> b c f", f=1),
                    p_allst[:, :, 1:2], ALU.mult)
                St = sp.tile([128, 1], F32, tag="St", name=f"St{su}")
                nc.vector.reduce_sum(St[:], tt16[:], axis=AX.X)
                lnS = sp.tile([128, 1], F32, tag="lnS", name=f"lnS{su}")
                nc.scalar.activation(lnS[:], St[:], AF.Ln)
                # dummy tanh swaps the ACT table back while off-critical
                nc.scalar.activation(dumm[:], dumm[:], AF.Tanh)
                lse = sp.tile([128, 1], F32, tag="lse", name=f"lse{su}")
                nc.vector.tensor_tensor(lse[:], p_Mt[:], lnS[:], ALU.add)
                outb = sp.tile([128, HALF], F32, tag="outb", bufs=1,
                               name=f"outb{su}")
                for q in range(4):
                    cs = slice(VCH * q, VCH * (q + 1))
                    nc.gpsimd.tensor_scalar(outb[:, cs], p_lg[:, cs],
                                            lse[:], None, op0=ALU.subtract)
                    dma_engs[q % 3].dma_start(
                        out_logp[:, p_t:p_t + 1, VCH * q:VCH * (q + 1)],
                        outb[0:B, cs].rearrange("b (x v) -> b x v", x=1))
                    dma_engs[(q + 1) % 3].dma_start(
                        out_logp[:, p_t:p_t + 1,
                                 HALF + VCH * q:HALF + VCH * (q + 1)],
                        outb[B:128, cs].rearrange("b (x v) -> b x v", x=1))

            prev = None
            h_sl, hT = h_sl0, hT0
            x_lo = x_hi = None

            for rep in range(REPS):
                if rep > 0:
                    h_sl, hT = h_sl0, hT0
                for t in range(T):
                    first = t == 0
                    su = f"{rep}_{t}"
                    want_tok = (t < T - 1) or (rep < REPS - 1)

                    # ==== GRU =============================================
                    rz_ps = pp.tile([B, 2 * HS], F32, tag="rz", name=f"rz{su}")
                    gin_ps = pp.tile([B, HS], F32, tag="gin", name=f"gin{su}")
                    ghn_ps = pp.tile([B, HS], F32, tag="ghn", name=f"ghn{su}")
                    nc.tensor.matmul(rz_ps[:], ones[:], brz[:], start=True,
                                     stop=False)
                    nc.tensor.matmul(gin_ps[:], ones[:], bin_[:], start=True,
                                     stop=False)
                    nc.tensor.matmul(ghn_ps[:], ones[:], bhn[:], start=True,
                                     stop=False)
                    for k in range(KC):
                        nc.tensor.matmul(rz_ps[:], hT[:, k, :],
                                         whh[:, k, 0:2 * HS],
                                         start=False, stop=False)
                        nc.tensor.matmul(ghn_ps[:], hT[:, k, :],
                                         whh[:, k, 2 * HS:GS],
                                         start=False, stop=(k == KC - 1))
                    for k in range(KC):
                        last = k == KC - 1
                        if first:
                            src = x0T[:, k, :]
                        else:
                            xsrc = x_lo if k < 4 else x_hi
                            xcol = (k % 4) * 128
                            xp_ps = pp.tile([128, B], F32, tag="xp",
                                            bufs=1 if SPLITPJ else 2,
                                            name=f"xp{su}_{k}")
                            nc.tensor.transpose(xp_ps[:],
                                                xsrc[:, xcol:xcol + 128],
                                                ident[0:B, 0:B])
                            xt_sb = sp.tile([128, B], F32, tag="xt", bufs=2,
                                            name=f"xt{su}_{k}")
                            nc.vector.tensor_copy(xt_sb[:], xp_ps[:])
                            src = xt_sb[:]
                        nc.tensor.matmul(rz_ps[:], src, wih[:, k, 0:2 * HS],
                                         start=False, stop=last)
                        nc.tensor.matmul(gin_ps[:], src, wih[:, k, 2 * HS:GS],
                                         start=False, stop=last)

                    # gates: sigmoid(v) = 0.5*(1+tanh(v/2)) -> all-tanh ACT
                    rz_sb = sp.tile([B, 2 * HS], F32, tag="rzsb", name=f"rzsb{su}")
                    nc.scalar.activation(rz_sb[:], rz_ps[:], AF.Tanh, scale=0.5)
                    t1 = sp.tile([B, HS], F32, tag="t1", name=f"t1_{su}")
                    nc.vector.scalar_tensor_tensor(t1[:], rz_sb[:, 0:HS], 1.0,
                                                   ghn_ps[:], op0=ALU.add,
                                                   op1=ALU.mult)
                    t2 = sp.tile([B, HS], F32, tag="t2", name=f"t2_{su}")
                    nc.vector.scalar_tensor_tensor(t2[:], t1[:], 0.5, gin_ps[:],
                                                   op0=ALU.mult, op1=ALU.add)
                    n_sb = sp.tile([B, HS], F32, tag="nsb", name=f"nsb{su}")
                    nc.scalar.activation(n_sb[:], t2[:], AF.Tanh)
                    t3 = sp.tile([B, HS], F32, tag="t3", name=f"t3_{su}")
                    nc.vector.tensor_tensor(t3[:], h_sl[:], n_sb[:], ALU.subtract)
                    t4 = sp.tile([B, HS], F32, tag="t4", name=f"t4_{su}")
                    nc.vector.scalar_tensor_tensor(t4[:], rz_sb[:, HS:2 * HS],
                                                   1.0, t3[:], op0=ALU.add,
                                                   op1=ALU.mult)
                    h_new = sp.tile([B, HS], F32, tag="hsl", name=f"hsl{su}")
                    nc.vector.scalar_tensor_tensor(h_new[:], t4[:], 0.5, n_sb[:],
                                                   op0=ALU.mult, op1=ALU.add)
                    h_sl = h_new

                    # ==== AllGather h (transposed) ========================
                    hxp_ps = pp.tile([128, B], F32, tag="xp",
                                     bufs=1 if SPLITPJ else 2,
                                     name=f"hxp{su}")
                    nc.tensor.transpose(hxp_ps[:], h_sl[:], ident[0:B, 0:B])
                    hTo = sp.tile([128, B], F32, tag="hTo", name=f"hTo{su}")
                    nc.vector.tensor_copy(hTo[:], hxp_ps[:])
                    ch_in = dp.tile([128, B], F32, tag="chin", name=f"chin{su}")
                    nc.sync.dma_start(ch_in[:], hTo[:])
                    ch_out = dp.tile([KC * 128, B], F32, tag="chout",
                                     addr_space="Shared", name=f"chout{su}")
                    all_gather(ch_in, ch_out)
                    hT_new = sp.tile([128, KC, B], F32, tag="hT", name=f"hT{su}")
                    for k in range(KC):
                        dma_engs[k % 3].dma_start(
                            hT_new[:, k, :], ch_out[128 * k:128 * (k + 1), :])
                    hT = hT_new

                    # ==== deferred log_softmax + writeback of step t-1 ====
                    if prev is not None:
                        writeback(prev, su)

                    # ==== projection (col-tiled, 2 passes of 2 chunks) ====
                    lg = sp.tile([128, HALF], F32, tag="lg", bufs=1,
                                 name=f"lg{su}")
                    mxs, idxs = [], []
                    for p in range(2):
                        js = (2 * p, 2 * p + 1)
                        tiles = {}
                        for j in js:
                            pjA = pp.tile([128, VCH], F32, tag=f"pj{j % 2}",
                                          name=f"pj{su}_{j}")
                            if SPLITPJ:
                                pjB = pp.tile([128, VCH], F32,
                                              tag=f"qj{j % 2}",
                                              name=f"qj{su}_{j}")
                            else:
                                pjB = pjA
                            tiles[j] = (pjA, pjB)
                            nc.tensor.matmul(pjA[0:B, :], ones[:],
                                             bo[:, VCH * j:VCH * (j + 1)],
                                             start=True,
                                             stop=NOPROJ,
                                             tile_position=(0, 0))
                            nc.tensor.matmul(
                                pjB[B:128, :], ones[:],
                                bo[:, HALF + VCH * j:HALF + VCH * (j + 1)],
                                start=True, stop=NOPROJ, tile_position=(0, 64))
                        for k in range(KC if not NOPROJ else 0):
                            last = k == KC - 1
                            for j in js:
                                pjA, pjB = tiles[j]
                                nc.tensor.matmul(
                                    pjA[0:B, :], hT[:, k, :],
                                    wot[:, k, VCH * j:VCH * (j + 1)],
                                    start=False, stop=last,
                                    tile_position=(0, 0))
                                nc.tensor.matmul(
                                    pjB[B:128, :], hT[:, k, :],
                                    wot[:, k, HALF + VCH * j:HALF + VCH * (j + 1)],
                                    start=False, stop=last,
                                    tile_position=(0, 64))
                        for i, j in enumerate(js):
                            pjA, pjB = tiles[j]
                            eng = nc.scalar if i == 0 else nc.vector
                            if SPLITPJ:
                                eng.tensor_copy(lg[0:B, VCH * j:VCH * (j + 1)],
                                                pjA[0:B, :]) \
                                    if i != 0 else nc.scalar.copy(
                                        lg[0:B, VCH * j:VCH * (j + 1)],
                                        pjA[0:B, :])
                                (nc.vector.tensor_copy if i == 0
                                 else nc.scalar.copy)(
                                    lg[B:128, VCH * j:VCH * (j + 1)],
                                    pjB[B:128, :])
                            elif i == 0:
                                nc.scalar.copy(lg[:, VCH * j:VCH * (j + 1)],
                                               pjA[:])
                            else:
                                nc.vector.tensor_copy(
                                    lg[:, VCH * j:VCH * (j + 1)], pjA[:])
                        # per-pass top8(+index): pass-0 stats hide under
                        # pass-1 matmuls
                        mx = sp.tile([128, 8], F32, tag=f"mx{p}",
                                     name=f"mx{su}_{p}")
                        nc.vector.max(mx[:], lg[:, 1000 * p:1000 * (p + 1)])
                        mxs.append(mx)
                        if want_tok:
                            mi = sp.tile([128, 8], U32, tag=f"mi{p}",
                                         name=f"mi{su}_{p}")
                            nc.vector.max_index(mi[:], mx[:],
                                                lg[:, 1000 * p:1000 * (p + 1)])
                            idxs.append(mi)

                    # ==== local stats =====================================
                    stats = sp.tile([128, 4], F32, tag="stats", name=f"stats{su}")
                    nc.vector.tensor_tensor(stats[:, 0:1], mxs[0][:, 0:1],
                                            mxs[1][:, 0:1], ALU.max)
                    neg_m = sp.tile([128, 1], F32, tag="negm", name=f"negm{su}")
                    nc.vector.tensor_scalar_mul(neg_m[:], stats[:, 0:1], -1.0)
                    s4 = sp.tile([128, 4], F32, tag="s4", name=f"s4_{su}")
                    for j in range(4):
                        sink = pp.tile([128, VCH], F32, tag=f"pj{j % 2}",
                                       name=f"esink{su}_{j}")
                        nc.scalar.activation(sink[:], lg[:, VCH * j:VCH * (j + 1)],
                                             AF.Exp, bias=neg_m[:],
                                             accum_out=s4[:, j:j + 1])
                    nc.vector.reduce_sum(stats[:, 1:2], s4[:], axis=AX.X)
                    if want_tok:
                        i0 = sp.tile([128, 1], F32, tag="i0", name=f"i0{su}")
                        nc.vector.tensor_copy(i0[:], idxs[0][:, 0:1])
                        i1 = sp.tile([128, 1], F32, tag="i1", name=f"i1{su}")
                        nc.vector.tensor_copy(i1[:], idxs[1][:, 0:1])
                        nc.vector.tensor_scalar_add(i1[:], i1[:], 1000.0)
                        gtm = sp.tile([128, 1], U8, tag="gtm", name=f"gtm{su}")
                        nc.vector.tensor_tensor(gtm[:], mxs[1][:, 0:1],
                                                mxs[0][:, 0:1], ALU.is_gt)
                        nc.vector.select(stats[:, 2:3], gtm[:], i1[:], i0[:])
                        nc.vector.tensor_tensor(stats[:, 2:3], stats[:, 2:3],
                                                voff[:], ALU.add)

                    # ==== exchange stats ==================================
                    cs_in = dp.tile([128, 4], F32, tag="csin", name=f"csin{su}")
                    nc.sync.dma_start(cs_in[:], stats[:])
                    cs_out = dp.tile([NCORE * 128, 4], F32, tag="csout",
                                     addr_space="Shared", name=f"csout{su}")
                    all_gather(cs_in, cs_out)
                    allst = sp.tile([128, 2 * NCORE, 4], F32, tag="allst",
                                    name=f"allst{su}")
                    rd = cs_out[:].rearrange("(c h b) f -> b (c h) f",
                                             c=NCORE, h=2)
                    nc.sync.dma_start(allst[0:B, :, :], rd)
                    nc.scalar.dma_start(allst[B:128, :, :], rd)

                    # ==== global max + winner token =======================
                    NE = 2 * NCORE
                    Mt = sp.tile([128, 1], F32, tag="Mt", name=f"Mt{su}")
                    nc.vector.tensor_reduce(Mt[:], allst[:, :, 0:1],
                                            axis=AX.XY, op=ALU.max)
                    if want_tok:
                        mask16 = sp.tile([128, NE], U8, tag="mask16",
                                         name=f"mask16{su}")
                        nc.vector.tensor_scalar(
                            mask16[:].rearrange("b (c f) -> b c f", f=1),
                            allst[:, :, 0:1], Mt[:], None, op0=ALU.is_equal)
                        c16 = sp.tile([128, NE], F32, tag="c16", name=f"c16{su}")
                        nc.vector.select(
                            c16[:].rearrange("b (c f) -> b c f", f=1),
                            mask16[:].rearrange("b (c f) -> b c f", f=1),
                            allst[:, :, 2:3],
                            bigc[:].rearrange("b (c f) -> b c f", f=1))
                        tokf = sp.tile([128, 1], F32, tag="tokf", name=f"tokf{su}")
                        nc.vector.tensor_reduce(tokf[:], c16[:], axis=AX.X,
                                                op=ALU.min)
                        tokf2 = sp.tile([128, 1], F32, tag="tokf2",
                                        name=f"tokf2{su}")
                        nc.vector.tensor_scalar_mul(tokf2[:], tokf[:], 2.0)
                        tok_u = sp.tile([128, 1], U32, tag="toku",
                                        name=f"toku{su}")
                        nc.vector.tensor_copy(tok_u[:], tokf2[:])
                        # emb viewed as [2V, 512]: row 2t = first half of
                        # token t's embedding, row 2t+1 = second half
                        emb2 = emb_t[:, :].rearrange("v (a c) -> (v a) c", a=2)
                        x_lo = sp.tile([B, H // 2], F32, tag="xlo", bufs=1,
                                       name=f"xlo{su}")
                        nc.gpsimd.indirect_dma_start(
                            x_lo[:], None, emb2,
                            IndirectOffsetOnAxis(ap=tok_u[0:B, :], axis=0),
                            bounds_check=2 * V - 1, oob_is_err=False)
                        x_hi = sp.tile([B, H // 2], F32, tag="xhi", bufs=1,
                                       name=f"xhi{su}")
                        nc.gpsimd.indirect_dma_start(
                            x_hi[:], None, emb2,
                            IndirectOffsetOnAxis(ap=tok_u[0:B, :], axis=0),
                            element_offset=H // 2,
                            bounds_check=2 * V - 1, oob_is_err=False)

                    prev = (lg, allst, Mt, t)

            # ---- final writeback + hidden state ---------------------------
            writeback(prev, "F")
            nc.sync.dma_start(out_hsl[:, :], h_sl[:])

    nc.compile()
    return nc


def _prep_core_inputs(c, encoder_hidden, emb, W_ih, W_hh, b_ih, b_hh, W_out,
                      b_out):
    HSl = slice(c * HS, (c + 1) * HS)
    gate_rows = np.r_[c * HS:(c + 1) * HS,
                      H + c * HS:H + (c + 1) * HS,
                      2 * H + c * HS:2 * H + (c + 1) * HS]
    rz_rows = gate_rows[:2 * HS]
    n_rows = gate_rows[2 * HS:]

    def to_kpb(a):  # [rows, H] -> [128, KC, rows] stationary layout
        return np.ascontiguousarray(
            a.T.reshape(KC, 128, a.shape[0]).transpose(1, 0, 2))

    h0 = encoder_hidden[0]
    x0 = np.broadcast_to(emb[SOS], (B, H))
    return {
        "emb": np.ascontiguousarray(emb),
        "wot": to_kpb(W_out[c * VS:(c + 1) * VS]),
        "wih": to_kpb(W_ih[gate_rows]),
        "whh": to_kpb(W_hh[gate_rows]),
        "bo": b_out[c * VS:(c + 1) * VS][None, :],
        "brz": (b_ih[rz_rows] + b_hh[rz_rows])[None, :],
        "bin": b_ih[n_rows][None, :],
        "bhn": b_hh[n_rows][None, :],
        "h0sl": np.ascontiguousarray(h0[:, HSl]),
        "h0T": np.ascontiguousarray(h0.T.reshape(KC, 128, B).transpose(1, 0, 2)),
        "x0T": np.ascontiguousarray(x0.T.reshape(KC, 128, B).transpose(1, 0, 2)),
        "ident": np.eye(128, dtype=np.float32),
        "voff": (c * VS + 2000.0 * (np.arange(128) >= 64)).astype(np.float32)[:, None],
    }


def kernel(encoder_outputs, encoder_hidden, emb, W_ih, W_hh, b_ih, b_hh,
           W_out, b_out, _trace=False):
    del encoder_outputs  # unused by the reference decoder (no attention)
    args = [np.asarray(a, dtype=np.float32) for a in
            (encoder_hidden, emb, W_ih, W_hh, b_ih, b_hh, W_out, b_out)]

    if "nc" not in _CACHE:
        _CACHE["nc"] = _build()
    nc = _CACHE["nc"]

    in_maps = [_prep_core_inputs(c, *args) for c in range(NCORE)]
    res = bass_utils.run_bass_kernel_spmd(
        nc, in_maps, core_ids=list(range(NCORE)), trace=_trace)

    log_probs = np.concatenate(
        [res.results[c]["out_logp"] for c in range(NCORE)], axis=2)
    hidden = np.concatenate(
        [res.results[c]["out_hsl"] for c in range(NCORE)], axis=1)[None]
    if _trace:
        _CACHE["last_result"] = res
    return log_probs, hidden
